# revision 6
# baseline (speedup 1.0000x reference)
"""Conv3d(32->64, k=3, pad=1) + BatchNorm(training) + LeakyReLU(0.2) on
(2, 32, 96, 96, 35), distributed over 8 TRN2 NeuronCores.

Strategy:
  - Shard H (96 = 8 x 12 rows per core). Halo rows + spatial zero-padding are
    materialized host-side into a per-core tensor xs of shape (2,32,14,98,39)
    (1 zero row each side of the 12-row H shard; W padded 96->98; D padded
    35->39 so that three d-shifted SBUF copies can be loaded by shifted reads).
  - Conv as implicit GEMM in bf16 (f32 PSUM accumulate): contraction
    K = 96 = C_in(32) x kd(3).  The SBUF "slab" for one input row holds 3
    partition-groups, group j pre-shifted by j elements along D.  Each of the
    9 (kh,kw) taps is then one matmul whose rhs is a 2-free-dim (w,d) window
    of the slab (12 w-cols x 35 d, the 2 padded d columns are skipped by the
    strided moving AP); kd is folded into the contraction.  PSUM accumulates.
  - M = C_out = 64 uses half the PE columns, so two spatial w-tiles run as a
    column-pair: tile A -> psum[0:64], tile B -> psum[64:128] (distinct PE
    column groups overlap in hardware).
  - BatchNorm (training stats): bn_stats per evicted tile -> bn_aggr ->
    (sum, sumsq) -> AllReduce over the 8 cores.  The reduce is SPLIT: blocks
    0..17 are aggregated and all-reduced while the conv of blocks 18..23 is
    still running (hides collective latency + inter-core skew); a second
    small AllReduce covers the tail.  The conv bias b cancels in training-
    mode BN and is unused.
  - Pass 2 applies scale/shift + LeakyReLU as one parametric-relu activation
    writing fp16, stored to DRAM in the SBUF-native [128, 40320] layout with
    large contiguous descriptors; the host unscrambles to (B,C,HS,W,D) and
    casts to fp32.
"""

import numpy as np
import ml_dtypes

import concourse.bacc as bacc
import concourse.bass as bass
import concourse.tile as tile
from concourse import mybir
from concourse.bass_utils import run_bass_kernel_spmd

N_CORES = 8
B, C_IN, C_OUT = 2, 32, 64
H, W, D = 96, 96, 35
HS = H // N_CORES          # 12 output rows per core
HR = HS + 2                # 14 input rows (halo)
WP, DP = W + 2, D + 2      # padded W / padded D for the host tensor
RW = D + 2                 # 37: slab row width per w-column (full padded D)
SLAB = WP * RW + 2 + 30    # slab row extent incl. group-shift + junk-read slack
WT = 12                    # w-tile width (8 uniform tiles)
NF = WT * D                # 420 matmul free size (strided moving AP skips pad)
BLK = B * HS               # 24 (b,h) blocks per core
BLKCOLS = 4 * NF           # 1680 conv-buffer columns per block per half
NREC = BLK * 4             # 96 bn_stats records per partition
SPLIT = 18                 # blocks in the first (overlapped) stats AllReduce
CNT1 = float(SPLIT * 4 * NF)           # elements per partition, blocks 0..17
CNT2 = float((BLK - SPLIT) * 4 * NF)   # elements per partition, blocks 18..23
N_TOT = float(B * H * W * D)           # 645120
EPS = 1e-5
NEG = 0.2

F32 = mybir.dt.float32
F16 = mybir.dt.float16
BF16 = mybir.dt.bfloat16
NP_BF16 = ml_dtypes.bfloat16

_CACHE = {}


def _build():
    nc = bacc.Bacc("TRN2", target_bir_lowering=False, debug=False,
                   num_devices=N_CORES)
    xs = nc.dram_tensor("xs", [B, C_IN, HR, WP, DP], BF16, kind="ExternalInput")
    wt = nc.dram_tensor("wt", [3, 3, 96, C_OUT], BF16, kind="ExternalInput")
    gm = nc.dram_tensor("gm", [C_OUT], F32, kind="ExternalInput")
    bt = nc.dram_tensor("bt", [C_OUT], F32, kind="ExternalInput")
    ys = nc.dram_tensor("ys", [128, BLK * BLKCOLS], F16, kind="ExternalOutput")

    xs_ap = xs.ap()
    ys_ap = ys.ap()

    from contextlib import ExitStack
    with tile.TileContext(nc) as tc:
        with tc.tile_pool(name="singles", bufs=1) as singles, \
             tc.tile_pool(name="dram", bufs=1, space="DRAM") as dramp:
            phase1 = ExitStack()
            slabp = phase1.enter_context(tc.tile_pool(name="slab", bufs=4))
            psump = phase1.enter_context(
                tc.tile_pool(name="psum", bufs=4, space="PSUM"))

            # ---- one-time loads ----
            wtile = singles.tile([96, 9, C_OUT], BF16)
            nc.sync.dma_start(
                out=wtile,
                in_=wt.ap().rearrange("kh kw p o -> p (kh kw) o"))
            gmt = singles.tile([C_OUT, 1], F32)
            nc.sync.dma_start(out=gmt, in_=gm.ap().rearrange("(p o) -> p o", o=1))
            btt = singles.tile([C_OUT, 1], F32)
            nc.sync.dma_start(out=btt, in_=bt.ap().rearrange("(p o) -> p o", o=1))

            cb = singles.tile([128, BLK * BLKCOLS], BF16)   # conv results
            st = singles.tile([128, NREC * 6], F32)          # bn_stats records

            # stats staging (first reduce overlaps the conv tail)
            mv1 = singles.tile([128, 2], F32)
            sq1 = singles.tile([128, 2], F32)
            t1a = singles.tile([128, 1], F32)
            mv2 = singles.tile([128, 2], F32)
            sq2 = singles.tile([128, 2], F32)
            t2a = singles.tile([128, 1], F32)
            cc1_in = dramp.tile([128, 2], F32)
            cc1_out = dramp.tile([128, 2], F32)
            cc2_in = dramp.tile([128, 2], F32)
            cc2_out = dramp.tile([128, 2], F32)
            gl1 = singles.tile([128, 2], F32)
            gl2 = singles.tile([128, 2], F32)

            def stats_reduce(lo, hi, cntf, mv, sq, tmp, cc_in, cc_out, gl):
                nc.vector.bn_aggr(
                    out=mv,
                    in_=st.rearrange("p (r s) -> p r s", s=6)[:, lo:hi, :])
                # sum = mean * n ; sumsq = (var + mean^2) * n
                nc.vector.tensor_scalar_mul(sq[:, 0:1], mv[:, 0:1], cntf)
                nc.vector.tensor_mul(tmp, mv[:, 0:1], mv[:, 0:1])
                nc.vector.tensor_add(tmp, tmp, mv[:, 1:2])
                nc.vector.tensor_scalar_mul(sq[:, 1:2], tmp, cntf)
                nc.gpsimd.dma_start(out=cc_in[:, :], in_=sq)
                nc.gpsimd.collective_compute(
                    "AllReduce", mybir.AluOpType.add,
                    replica_groups=[list(range(N_CORES))],
                    ins=[cc_in[:, :].opt()], outs=[cc_out[:, :].opt()])
                nc.gpsimd.dma_start(out=gl, in_=cc_out[:, :])

            # ---- pass 1: conv + stats ----
            # slab group tiles hold 2 input rows (rows 2g, 2g+1)
            for b in range(B):
                groups = {}
                for h in range(HS):
                    for r in (h, h + 1, h + 2):
                        g = r // 2
                        if g not in groups:
                            gt = slabp.tile([96, 2, SLAB], BF16, tag="slab")
                            for j in range(3):
                                # group j holds the full row shifted by (2-j)
                                nc.sync.dma_start(
                                    out=gt[32 * j:32 * (j + 1), :,
                                           2 - j:2 - j + WP * RW],
                                    in_=xs_ap[b, :, 2 * g:2 * g + 2, :, :].rearrange(
                                        "p r w d -> p r (w d)"))
                            groups[g] = gt
                    blk = b * HS + h
                    for k in range(4):
                        w0a = 2 * k * WT
                        w0b = (2 * k + 1) * WT
                        ps = psump.tile([128, NF], F32, tag="ps")
                        for kh in range(3):
                            r = h + kh
                            gt = groups[r // 2]
                            rs = r % 2
                            row = gt[:, rs, 2:2 + WP * RW].rearrange(
                                "p (w d) -> p w d", d=RW)
                            for kw in range(3):
                                q = kh * 3 + kw
                                first, last = q == 0, q == 8
                                wa = w0a + kw
                                wb = w0b + kw
                                # 2-free-dim moving AP: 12 w-cols (stride RW)
                                # x 35 d (row is based at the +2 shift origin)
                                nc.tensor.matmul(
                                    ps[0:64, :],
                                    lhsT=wtile[:, q, :],
                                    rhs=row[:, wa:wa + WT, 0:D],
                                    start=first, stop=last)
                                nc.tensor.matmul(
                                    ps[64:128, :],
                                    lhsT=wtile[:, q, :],
                                    rhs=row[:, wb:wb + WT, 0:D],
                                    start=first, stop=last)
                        # evict + stats (both contiguous now)
                        col = blk * BLKCOLS + k * NF
                        rec = (blk * 4 + k) * 6
                        nc.scalar.copy(out=cb[:, col:col + NF], in_=ps)
                        nc.vector.bn_stats(out=st[:, rec:rec + 6],
                                           in_=cb[:, col:col + NF])
                    if blk == SPLIT - 1:
                        # first stats reduce rides under the conv tail
                        stats_reduce(0, SPLIT * 4, CNT1,
                                     mv1, sq1, t1a, cc1_in, cc1_out, gl1)

            stats_reduce(SPLIT * 4, NREC, CNT2,
                         mv2, sq2, t2a, cc2_in, cc2_out, gl2)

            phase1.close()

            # ---- combine stats + fold into scale/shift ----
            t128 = singles.tile([128, 2], F32)
            nc.vector.tensor_add(t128, gl1, gl2)
            hi = singles.tile([64, 2], F32)
            nc.sync.dma_start(out=hi, in_=t128[64:128, :])
            tot = singles.tile([64, 2], F32)
            nc.vector.tensor_add(tot, t128[0:64, :], hi)

            m_g = singles.tile([64, 1], F32)
            qn = singles.tile([64, 1], F32)
            var = singles.tile([64, 1], F32)
            sd = singles.tile([64, 1], F32)
            s64 = singles.tile([64, 1], F32)
            t64 = singles.tile([64, 1], F32)
            nc.vector.tensor_scalar_mul(m_g, tot[:, 0:1], 1.0 / N_TOT)
            nc.vector.tensor_scalar_mul(qn, tot[:, 1:2], 1.0 / N_TOT)
            nc.vector.tensor_mul(var, m_g, m_g)
            nc.vector.tensor_sub(var, qn, var)
            epst = singles.tile([64, 1], F32)
            nc.vector.memset(epst, EPS)
            nc.scalar.activation(out=sd, in_=var,
                                 func=mybir.ActivationFunctionType.Sqrt,
                                 bias=epst)
            nc.vector.reciprocal(out=sd, in_=sd)
            nc.vector.tensor_mul(s64, sd, gmt)      # s = gamma * rsqrt(var+eps)
            nc.vector.tensor_mul(t64, m_g, s64)
            nc.vector.tensor_sub(t64, btt, t64)     # t = beta - mean * s

            s_all = singles.tile([128, 1], F32)
            t_all = singles.tile([128, 1], F32)
            nc.vector.tensor_copy(s_all[0:64, :], s64)
            nc.vector.tensor_copy(t_all[0:64, :], t64)
            nc.sync.dma_start(out=s_all[64:128, :], in_=s_all[0:64, :])
            nc.sync.dma_start(out=t_all[64:128, :], in_=t_all[0:64, :])

            # ---- pass 2: normalize + LeakyReLU + contiguous fp16 writeback ----
            stgp = phase1.enter_context(tc.tile_pool(name="stg", bufs=3))
            CH = 4  # blocks per chunk
            rings = [nc.sync, nc.scalar, nc.gpsimd]
            for i, blk in enumerate(range(0, BLK, CH)):
                c0 = blk * BLKCOLS
                c1 = (blk + CH) * BLKCOLS
                stg = stgp.tile([128, CH * BLKCOLS], F16, tag="stg")
                nc.scalar.activation(
                    out=stg, in_=cb[:, c0:c1],
                    func=mybir.ActivationFunctionType.Prelu,
                    bias=t_all, scale=s_all, alpha=NEG)
                rings[i % 3].dma_start(out=ys_ap[:, c0:c1], in_=stg)

            phase1.close()
    nc.finalize()
    return nc


def _get_nc():
    if "nc" not in _CACHE:
        _CACHE["nc"] = _build()
    return _CACHE["nc"]


def _prep(x, w, gamma, beta):
    xpad = np.zeros((B, C_IN, H + 2, WP, DP), dtype=np.float32)
    xpad[:, :, 1:H + 1, 1:W + 1, 1:D + 1] = x
    wt = np.ascontiguousarray(
        np.asarray(w, dtype=np.float32).transpose(2, 3, 4, 1, 0).reshape(
            3, 3, 96, C_OUT)).astype(NP_BF16)
    gm = np.ascontiguousarray(np.asarray(gamma, dtype=np.float32))
    bt = np.ascontiguousarray(np.asarray(beta, dtype=np.float32))
    in_maps = []
    for c in range(N_CORES):
        xsl = np.ascontiguousarray(
            xpad[:, :, c * HS:c * HS + HR, :, :]).astype(NP_BF16)
        in_maps.append({"xs": xsl, "wt": wt, "gm": gm, "bt": bt})
    return in_maps


def kernel(x, w, b, gamma, beta):
    nc = _get_nc()
    in_maps = _prep(np.asarray(x, dtype=np.float32), w, gamma, beta)
    res = run_bass_kernel_spmd(nc, in_maps, core_ids=list(range(N_CORES)))
    out = np.empty((B, C_OUT, H, W, D), dtype=np.float32)
    for c in range(N_CORES):
        # ys rows: [half(2) x ch(64)]; cols: [b(2) x h(12) x k(4) x w'(12) x d(35)]
        a = res.results[c]["ys"].reshape(2, C_OUT, B, HS, 4, WT, D)
        # w = k*24 + half*12 + w'
        out[:, :, c * HS:(c + 1) * HS] = a.transpose(
            2, 1, 3, 4, 0, 5, 6).reshape(B, C_OUT, HS, W, D)
    return out
